# revision 63
# baseline (speedup 1.0000x reference)
"""Multi-head attention (B=4, S=2048, E=768, H=12, Dh=64) on 8 TRN2 NeuronCores.

Sharding: batch x head-group tensor parallel. Core c handles batch b = c//2 and
head group g = c%2 (6 heads each). Each core computes its heads' Q/K/V
projections, full attention over the 2048-token sequence, and a partial
out-projection over its 384 concat-features. The host sums the two partials per
batch and adds the output bias.

Device layout notes:
 - Host pre-transposes activations to x^T [E, S] and casts to bf16, so the
   contraction dim (E) lands on SBUF partitions with contiguous DMA loads.
   Each activation lives in ONE SBUF tile [128, 6, T] filled by a few large
   column-group DMAs (few HWDGE descriptors, deep overlap with compute),
   ordered so each PE phase's gating tensor arrives just in time.
 - Q^T/K^T are produced feature-major [384, S]; Q/K biases are folded into
   the PSUM->SBUF copy (per-partition scalar: ScalarE Identity-with-bias in
   the pre-attention phase while it is idle, DVE tensor_scalar after), so
   there are no PE bias matmuls on the Q/K path. The V bias is broadcast
   once to a [128, 384] tile and added during V's PSUM->SBUF copy.
 - V is token-major, each head augmented with 64 ones columns so the PV
   matmul emits the softmax denominator replicated on psum partitions
   64-127 (normalization is then one DVE reciprocal + one multiply-cast).
 - Scores are computed transposed (S^T tiles [128 keys, S queries]) and
   exponentiated on ScalarE straight out of PSUM (no max-subtraction: logits
   are ~N(0, 0.3), so exp is numerically safe, matching softmax exactly).
 - PSUM (8 banks) holds proj (2) + double-buffered S^T (4) + ctx (2).
 - The exp stream on ScalarE is the critical path (~200us busy): each of
   the 12 head-half windows emits only exp + score-tile matmuls at exp
   pace; ALL other PE work (context matmuls, projections, out-projection)
   flows through a deferred FIFO drained ~550ns per step with carried
   budget. A 16-deep pt pool lets the context chain lag the exp stream by
   up to 14 steps, so bursts of dependency-forced work (window 0 carries
   the whole DMA tail) spread over later windows instead of stalling exp.
   Items are emitted in two ~0.6us halves sharing one open PSUM group;
   chunk deadlines force projection halves out before the score tiles that
   read them (emission order defines Tile dependencies - see emit_audit).
 - Sweep order: all pairs' first query half, then all second halves, so
   the first-half out-projection spreads over the sh=1 sweep; the last
   head's normalize is split 4 ways so the tail out-projection pipelines
   with it, with its copies alternating DVE/ScalarE (idle by then).
 - A few ones x ones warmup matmuls ramp the PE p-state during the initial
   DMA wait so real work starts at full clock.
"""

import math
import os
import sys
from contextlib import ExitStack

import numpy as np

for _p in ("/opt/trn_rl_repo", "/root/.axon_site/_ro/trn_rl_repo"):
    if os.path.isdir(_p) and _p not in sys.path:
        sys.path.append(_p)

# NTFF tracing hooks (antenv.axon_hooks) don't exist in this container;
# make sure an ambient BASS_TRACE can't route execution into that path.
os.environ["BASS_NEVER_TRACE"] = "1"

import ml_dtypes  # noqa: E402

import concourse.bass as bass  # noqa: E402
import concourse.tile as tile  # noqa: E402
from concourse import bacc, mybir  # noqa: E402
from concourse.bass_utils import run_bass_kernel_spmd  # noqa: E402

BF16 = mybir.dt.bfloat16
F32 = mybir.dt.float32
NP_BF16 = ml_dtypes.bfloat16

B, S, E, H, DH = 4, 2048, 768, 12, 64
N_CORES = 8
G = H // 2  # heads per core (6)


def build_nc(T=S, EMB=E, NH=G, dh=DH, OUT=E, trace_label=""):
    """Emit the per-core Bass/Tile program. All cores run this same program.

    T: sequence length; EMB: model dim; NH: heads on this core (even);
    dh: head dim (64); OUT: out-projection output width.
    """
    assert T % 128 == 0 and EMB % 128 == 0 and dh == 64 and NH % 2 == 0
    FEAT = NH * dh
    assert FEAT % 128 == 0
    EC = EMB // 128  # contraction chunks for projections
    TT = T // 128  # token tiles
    FT = FEAT // 128  # feature tiles (head pairs)
    SCH = min(512, T)  # matmul moving free-dim chunk
    NSCH = T // SCH
    T2 = max(128, T // 2)  # attention query-half width (2 PSUM banks)
    NSH = T // T2  # query halves per head
    SCH2 = min(512, T2)
    NSCH2 = T2 // SCH2
    _ock = OUT // 2 if 128 < OUT <= 1024 and OUT % 2 == 0 else 512
    OCHUNKS = [(o, min(_ock, OUT - o)) for o in range(0, OUT, _ock)]
    XG = min(512, T)  # x DMA column-group width
    NXG = T // XG
    scale = 1.0 / math.sqrt(dh)

    nc = bacc.Bacc("TRN2", target_bir_lowering=False, debug=False, num_devices=N_CORES)

    # ---- DRAM I/O ----
    xqT_d = nc.dram_tensor("xqT", [EMB, T], BF16, kind="ExternalInput").ap()
    xkT_d = nc.dram_tensor("xkT", [EMB, T], BF16, kind="ExternalInput").ap()
    xvT_d = nc.dram_tensor("xvT", [EMB, T], BF16, kind="ExternalInput").ap()
    wq_d = nc.dram_tensor("wq", [EMB, FEAT], BF16, kind="ExternalInput").ap()
    wk_d = nc.dram_tensor("wk", [EMB, FEAT], BF16, kind="ExternalInput").ap()
    wv_d = nc.dram_tensor("wv", [EMB, FEAT], BF16, kind="ExternalInput").ap()
    wo_d = nc.dram_tensor("wo", [FEAT, OUT], BF16, kind="ExternalInput").ap()
    # q/k biases feature-tile-major [128, FT] fp32 (per-partition scalars for
    # the DVE tensor_scalar bias-add); v bias stays [1, FEAT] for its rank-1
    # matmul.
    bq_d = nc.dram_tensor("bq", [128, FT], F32, kind="ExternalInput").ap()
    bk_d = nc.dram_tensor("bk", [128, FT], F32, kind="ExternalInput").ap()
    bv_d = nc.dram_tensor("bv", [1, FEAT], BF16, kind="ExternalInput").ap()
    out_d = nc.dram_tensor("out", [T, OUT], BF16, kind="ExternalOutput").ap()

    def dram3(ap_, width=None):
        # [EMB, W] -> [128, EC, W] view (partition, e-chunk, col)
        v = ap_ if width is None else ap_
        return v.rearrange("(c p) t -> p c t", p=128)

    with tile.TileContext(nc) as tc, ExitStack() as ctx:
        persist = ctx.enter_context(tc.tile_pool(name="persist", bufs=1))

        # ---- persistent SBUF tensors ----
        wq_sb = persist.tile([128, EC, FEAT], BF16, tag="wq", name="wq")
        wk_sb = persist.tile([128, EC, FEAT], BF16, tag="wk", name="wk")
        wv_sb = persist.tile([128, EC, FEAT], BF16, tag="wv", name="wv")
        wo_sb = persist.tile([128, FT, OUT], BF16, tag="wo", name="wo")
        bq_sb = persist.tile([128, FT], F32, tag="bq", name="bq")
        bk_sb = persist.tile([128, FT], F32, tag="bk", name="bk")
        bv_sb = persist.tile([1, FEAT], BF16, tag="bv", name="bv")
        bvb_sb = persist.tile([128, FEAT], BF16, tag="bvb", name="bvb")
        ones_row = persist.tile([1, SCH], BF16, tag="ones_row", name="ones_row")
        xq_sb = persist.tile([128, EC, T], BF16, tag="xq", name="xq")
        xk_sb = persist.tile([128, EC, T], BF16, tag="xk", name="xk")
        xv_sb = persist.tile([128, EC, T], BF16, tag="xv", name="xv")
        qT_sb = [persist.tile([128, T], BF16, tag=f"qT{j}", name=f"qT{j}") for j in range(FT)]
        kT_sb = [persist.tile([128, T], BF16, tag=f"kT{j}", name=f"kT{j}") for j in range(FT)]
        # V token-major, each head augmented with 64 ones columns so the PV
        # matmul emits the softmax denominator replicated on partitions 64-127
        v_sb = [persist.tile([128, NH * (dh + 64)], BF16, tag=f"v{i}", name=f"v{i}") for i in range(TT)]
        cn_sb = [persist.tile([128, T], BF16, tag=f"cn{j}", name=f"cn{j}") for j in range(FT)]

        def xgroup(dst_sb, src_d, g):
            nc.sync.dma_start(
                dst_sb[:, :, g * XG : (g + 1) * XG],
                dram3(src_d)[:, :, g * XG : (g + 1) * XG],
            )

        # ---- DMA issue order == data-arrival priority ----
        # V path first (PE's first real work), then Q (all feature tiles of
        # the Q projection run while K streams), the xv tail, K, then the
        # out-projection weights.
        nc.sync.dma_start(bv_sb[:], bv_d[:])
        nc.sync.dma_start(wv_sb[:], dram3(wv_d))
        xgroup(xv_sb, xvT_d, 0)
        nc.sync.dma_start(wq_sb[:], dram3(wq_d))
        nc.sync.dma_start(bq_sb[:], bq_d[:])
        nc.sync.dma_start(bk_sb[:], bk_d[:])
        xgroup(xq_sb, xqT_d, 0)
        if NXG > 1:
            xgroup(xq_sb, xqT_d, 1)
        nc.sync.dma_start(wk_sb[:], dram3(wk_d))
        nc.sync.dma_start(
            xk_sb[:, :, 0 : XG // 2], dram3(xkT_d)[:, :, 0 : XG // 2]
        )
        nc.sync.dma_start(
            xk_sb[:, :, XG // 2 : XG], dram3(xkT_d)[:, :, XG // 2 : XG]
        )
        if NXG > 1:
            xgroup(xv_sb, xvT_d, 1)
        for g in range(1, NXG):
            xgroup(xk_sb, xkT_d, g)
        for g in range(2, NXG):
            xgroup(xv_sb, xvT_d, g)
        for g in range(2, NXG):
            xgroup(xq_sb, xqT_d, g)
        nc.sync.dma_start(wo_sb[:], wo_d[:].rearrange("(c p) t -> p c t", p=128))

        nc.vector.memset(ones_row[:], 1.0)
        # ones columns of augmented V (written once; on GpSimd so the DVE
        # stream isn't delayed ahead of the projection copies)
        for i in range(TT):
            vview = v_sb[i][:].rearrange("p (h x) -> p h x", x=dh + 64)
            nc.gpsimd.memset(vview[:, :, dh:], 1.0)

        # ---- compute: projections + attention + out-projection ----
        # PSUM budget (8 banks): proj 2 (bufs=2 x 1 bank) + ST 4 (bufs=2 x 2)
        # + ctx 2 (bufs=1 x 2). Everything coexists, so Tile can overlap the
        # phases; PE instruction order is software-pipelined by hand.
        with (
            tc.tile_pool(name="ppsum", bufs=2, space="PSUM") as ppool,
            tc.tile_pool(name="stpsum", bufs=2, space="PSUM") as stpool,
            tc.tile_pool(name="ctpsum", bufs=1, space="PSUM") as ctpool,
            tc.tile_pool(name="ptpool", bufs=16) as ptpool,
            tc.tile_pool(name="normpool", bufs=4) as npool,
            tc.tile_pool(name="outsb", bufs=4) as osbpool,
        ):

            def warmup(n):
                # data-independent matmuls (gated only on the ones_row memset)
                # that ramp the PE p-state while the first DMAs are in flight
                for _ in range(n):
                    ps = ppool.tile([128, SCH], F32, tag="proj", name="warm")
                    nc.tensor.matmul(
                        ps[:], ones_row[:, 0:128], ones_row[:, 0:SCH],
                        start=True, stop=True,
                    )

            def proj_qk_group(t, j, n, act_copy=False, c0=0, cw=None):
                # one (tensor, feature-tile, token-chunk) projection group:
                # 6 accumulating matmuls + bias-folding copy to SBUF (on
                # ScalarE for pre-attention groups, while it is still idle)
                w_sb, b_sb, x_sb, dst = (
                    (wq_sb, bq_sb, xq_sb, qT_sb),
                    (wk_sb, bk_sb, xk_sb, kT_sb),
                )[t]
                cw = SCH if cw is None else cw
                lo = n * SCH + c0
                ps = ppool.tile([128, SCH], F32, tag="proj", name="proj")
                for e in range(EC):
                    nc.tensor.matmul(
                        ps[0:128, 0:cw],
                        w_sb[:, e, j * 128 : (j + 1) * 128],
                        x_sb[:, e, lo : lo + cw],
                        start=(e == 0),
                        stop=(e == EC - 1),
                    )
                if act_copy:
                    nc.scalar.activation(
                        dst[j][:, lo : lo + cw],
                        ps[0:128, 0:cw],
                        mybir.ActivationFunctionType.Identity,
                        bias=b_sb[:, j : j + 1],
                    )
                else:
                    nc.vector.tensor_scalar_add(
                        dst[j][:, lo : lo + cw], ps[0:128, 0:cw], b_sb[:, j : j + 1]
                    )

            def proj_v(i):
                ps = ppool.tile([128, FEAT], F32, tag="proj", name="proj")
                for e in range(EC):
                    nc.tensor.matmul(
                        ps[:],
                        xv_sb[:, e, i * 128 : (i + 1) * 128],
                        wv_sb[:, e, :],
                        start=(e == 0),
                        stop=(e == EC - 1),
                    )
                dst = v_sb[i][:].rearrange("p (h x) -> p h x", x=dh + 64)[:, :, 0:dh]
                nc.vector.tensor_tensor(
                    dst,
                    ps[:].rearrange("p (h d) -> p h d", d=dh),
                    bvb_sb[:].rearrange("p (h d) -> p h d", d=dh),
                    op=mybir.AluOpType.add,
                )

            def outproj(i, rows=slice(0, 128), pools=None, tail=False):
                r0, r1 = rows.start, rows.stop
                osb = osbpool.tile([128, OUT], BF16, tag="osb", name="osb")
                for ci, (oc, ow) in enumerate(OCHUNKS):
                    pool_c, tag_c = (
                        (ppool, "proj") if pools is None else pools[ci % len(pools)]
                    )
                    ps = pool_c.tile([128, ow], F32, tag=tag_c, name="proj")
                    for f in range(FT):
                        nc.tensor.matmul(
                            ps[0 : r1 - r0, :],
                            cn_sb[f][:, i * 128 + r0 : i * 128 + r1],
                            wo_sb[:, f, oc : oc + ow],
                            start=(f == 0),
                            stop=(f == FT - 1),
                        )
                    if tail and ci % 2 == 1:
                        nc.scalar.copy(osb[r0:r1, oc : oc + ow], ps[0 : r1 - r0, :])
                    else:
                        nc.vector.tensor_copy(osb[r0:r1, oc : oc + ow], ps[0 : r1 - r0, :])
                nc.sync.dma_start(out_d[i * 128 + r0 : i * 128 + r1, :], osb[r0:r1, :])

            # ---- deferred PE work queue ----
            # The exp stream is the critical path: it needs ONLY score tiles
            # (+ st-PSUM recycling, which is exp-paced). Everything else the
            # PE does — context matmuls, projections, out-projection — is
            # deferred through this FIFO and emitted a bounded budget per
            # step, so a burst of forced work (window 0's DMA tail) lags the
            # context chain by a few steps instead of stalling the exps.
            # The pt pool depth bounds the max context lag.
            dq = []  # entries: [cost_ns, deadline_gstep or None, thunk]
            gstep = [0]
            CT_COST = SCH2 * NSCH2 * 10 // 24  # ns per ct pair @2.4GHz
            QK_COST = EC * SCH * 10 // 24
            V_COST = (FEAT + EC * FEAT) * 10 // 24
            OUT_COST = OUT * FT * 10 // 24
            STEP_BUDGET = 620
            ct_pending = [0]
            CT_CAP = 14
            budget_acc = [0]
            carry = []

            def drain_dq(budget, flush=False):
                # carried budget: overshoot on a big item becomes debt, so
                # the long-run drain rate matches STEP_BUDGET exactly and a
                # burst of forced work spreads over following steps
                if flush:
                    for part in carry:
                        dq.append(part)
                    del carry[:]
                    while dq:
                        dq.pop(0)[2]()
                    return
                budget_acc[0] = min(budget_acc[0] + budget, 2 * STEP_BUDGET)
                while dq:
                    cost, deadline, fn = dq[0]
                    if not (
                        budget_acc[0] > 0
                        or (deadline is not None and deadline <= gstep[0])
                        or ct_pending[0] > CT_CAP
                    ):
                        break
                    dq.pop(0)
                    fn()
                    budget_acc[0] -= cost

            def item_thunks(item):
                # each queue item is split into two ~0.6us halves so context
                # matmuls interleave between them and the score-tile stream
                # never sits behind a full projection group. The halves of a
                # projection share one open PSUM accumulation group; the
                # scheduler keeps at most one non-deadline pair and one
                # deadline pair open at a time (= the 2 proj PSUM buffers).
                kind, arg = item
                EH = EC // 2
                if kind == "qk":
                    t, j, n = arg
                    w_sb, b_sb, x_sb, dst = (
                        (wq_sb, bq_sb, xq_sb, qT_sb),
                        (wk_sb, bk_sb, xk_sb, kT_sb),
                    )[t]
                    state = {}

                    def qa():
                        ps = ppool.tile([128, SCH], F32, tag="proj", name="proj")
                        state["ps"] = ps
                        for e in range(EH):
                            nc.tensor.matmul(
                                ps[:],
                                w_sb[:, e, j * 128 : (j + 1) * 128],
                                x_sb[:, e, n * SCH : (n + 1) * SCH],
                                start=(e == 0),
                                stop=False,
                            )

                    def qb():
                        ps = state["ps"]
                        for e in range(EH, EC):
                            nc.tensor.matmul(
                                ps[:],
                                w_sb[:, e, j * 128 : (j + 1) * 128],
                                x_sb[:, e, n * SCH : (n + 1) * SCH],
                                start=False,
                                stop=(e == EC - 1),
                            )
                        nc.vector.tensor_scalar_add(
                            dst[j][:, n * SCH : (n + 1) * SCH],
                            ps[:],
                            b_sb[:, j : j + 1],
                        )

                    return [(QK_COST // 2, qa), (QK_COST // 2, qb)]
                if kind == "v":
                    i = arg
                    state = {}

                    def va():
                        ps = ppool.tile([128, FEAT], F32, tag="proj", name="proj")
                        state["ps"] = ps
                        for e in range(EH):
                            nc.tensor.matmul(
                                ps[:],
                                xv_sb[:, e, i * 128 : (i + 1) * 128],
                                wv_sb[:, e, :],
                                start=(e == 0),
                                stop=False,
                            )

                    def vb():
                        ps = state["ps"]
                        for e in range(EH, EC):
                            nc.tensor.matmul(
                                ps[:],
                                xv_sb[:, e, i * 128 : (i + 1) * 128],
                                wv_sb[:, e, :],
                                start=False,
                                stop=(e == EC - 1),
                            )
                        dst = v_sb[i][:].rearrange("p (h x) -> p h x", x=dh + 64)[
                            :, :, 0:dh
                        ]
                        nc.vector.tensor_tensor(
                            dst,
                            ps[:].rearrange("p (h d) -> p h d", d=dh),
                            bvb_sb[:].rearrange("p (h d) -> p h d", d=dh),
                            op=mybir.AluOpType.add,
                        )

                    return [(V_COST // 2, va), (V_COST // 2, vb)]
                # out: the two output chunks are self-contained (each closes
                # its PSUM group); the DMA goes with the last one
                i = arg
                state = {}

                def mk_out(ci):
                    def f():
                        if ci == 0:
                            state["osb"] = osbpool.tile(
                                [128, OUT], BF16, tag="osb", name="osb"
                            )
                        osb = state["osb"]
                        oc, ow = OCHUNKS[ci]
                        ps = ppool.tile([128, ow], F32, tag="proj", name="proj")
                        for f_ in range(FT):
                            nc.tensor.matmul(
                                ps[:],
                                cn_sb[f_][:, i * 128 : (i + 1) * 128],
                                wo_sb[:, f_, oc : oc + ow],
                                start=(f_ == 0),
                                stop=(f_ == FT - 1),
                            )
                        nc.vector.tensor_copy(osb[:, oc : oc + ow], ps[:])
                        if ci == len(OCHUNKS) - 1:
                            nc.sync.dma_start(out_d[i * 128 : (i + 1) * 128, :], osb[:])

                    return f

                return [
                    (OUT_COST // len(OCHUNKS), mk_out(ci)) for ci in range(len(OCHUNKS))
                ]

            def st_tile(i, kT_h, qT_h, s0):
                st = stpool.tile([128, T2], F32, tag="st", name="st")
                for n in range(NSCH2):
                    nc.tensor.matmul(
                        st[:, n * SCH2 : (n + 1) * SCH2],
                        kT_h[:, i * 128 : (i + 1) * 128],
                        qT_h[:, s0 + n * SCH2 : s0 + (n + 1) * SCH2],
                        start=True,
                        stop=True,
                    )
                return st

            pending_sts = []

            def head_args(h, sh):
                ft, half = h // 2, (h % 2) * 64
                return (
                    kT_sb[ft][half : half + 64, :],
                    qT_sb[ft][half : half + 64, :],
                    sh * T2,
                )

            def head(h, sh, items=(), nxt=None, norm_chunks=1, budget=None):
                # emits the exp/score stream at ACT pace; context matmuls and
                # this window's drip items flow through the deferred queue.
                # `items`: [(step, deadline_rel or None, item), ...] —
                # deadline_rel forces emission by that step (projection
                # chunks that later score tiles read must be emitted first).
                ft, half = h // 2, (h % 2) * 64
                kT_h, qT_h, s0 = head_args(h, sh)
                ct = ctpool.tile([128, T2], F32, tag="ct", name="ct")
                sts = pending_sts[:]
                del pending_sts[:]
                while len(sts) < min(2, TT):
                    sts.append(st_tile(len(sts), kT_h, qT_h, s0))
                items = list(items)
                base = gstep[0]
                nissued = 0

                def ct_thunk(i, pt):
                    def fn():
                        for n in range(NSCH2):
                            nc.tensor.matmul(
                                ct[:, n * SCH2 : (n + 1) * SCH2],
                                v_sb[i][:, h * (dh + 64) : (h + 1) * (dh + 64)],
                                pt[:, n * SCH2 : (n + 1) * SCH2],
                                start=(i == 0),
                                stop=(i == TT - 1),
                            )
                        ct_pending[0] -= 1

                    return fn

                for i in range(TT):
                    st = sts.pop(0)
                    pt = ptpool.tile([128, T2], BF16, tag="pt", name="pt")
                    nc.scalar.activation(
                        pt[:], st[:], mybir.ActivationFunctionType.Exp, scale=scale
                    )
                    if i + 2 < TT:
                        sts.append(st_tile(i + 2, kT_h, qT_h, s0))
                    elif nxt is not None and nissued < min(2, TT):
                        pending_sts.append(st_tile(nissued, *head_args(*nxt)))
                        nissued += 1
                    # carried second halves from the previous step first, so
                    # a context matmul lands between the halves of a pair
                    for part in carry:
                        dq.append(part)
                    del carry[:]
                    while items and items[0][0] <= i:
                        _, dl, item = items.pop(0)
                        parts = item_thunks(item)
                        if dl is None:
                            dq.append([parts[0][0], None, parts[0][1]])
                            for c, fn in parts[1:]:
                                carry.append([c, None, fn])
                        else:
                            # deadline items (projection chunks read by later
                            # score tiles) have no FIFO hazard — jump the
                            # queue (halves adjacent) so forcing them never
                            # bursts the backlog
                            for c, fn in reversed(parts):
                                dq.insert(0, [c, base + dl, fn])
                    ct_pending[0] += 1
                    dq.append([CT_COST, None, ct_thunk(i, pt)])
                    gstep[0] += 1
                    drain_dq(STEP_BUDGET if budget is None else budget)

                # normalize: cn[f, s] = ct[f, s] * (1 / ct[64.., s]), chunked
                # for the last head so the tail out-projection starts early
                def norm_fn():
                    cw = T2 // norm_chunks
                    for c in range(norm_chunks):
                        recip = npool.tile([64, cw], F32, tag="recip", name="recip")
                        nc.vector.reciprocal(
                            recip[:], ct[64:128, c * cw : (c + 1) * cw]
                        )
                        nc.vector.tensor_tensor(
                            cn_sb[ft][half : half + 64, s0 + c * cw : s0 + (c + 1) * cw],
                            ct[0:64, c * cw : (c + 1) * cw],
                            recip[:],
                            op=mybir.AluOpType.mult,
                        )

                dq.append([0, None, norm_fn])

            # ---- emission schedule ----
            # Slim pre-phase matched to DMA arrivals: warmups bridge the
            # first transfers, V 0..7 streams with xv groups 0-1, Q j=0's
            # first query half with xq groups 0-1, then K j=0 chunk 0 (the
            # last gate for head 0's first score tiles). Everything else is
            # dripped into the attention windows below, so exp starts as
            # early as the data allows and window 0 absorbs the DMA tail.
            warmup(8)
            # one-time broadcast of the V bias to all 128 partitions: the
            # per-tile rank-1 bias matmuls become a single matmul + copy,
            # and each V tile's PSUM->SBUF copy turns into a bias add
            bps = ppool.tile([128, FEAT], F32, tag="proj", name="proj")
            nc.tensor.matmul(bps[:], ones_row[:, 0:128], bv_sb[:], start=True, stop=True)
            nc.vector.tensor_copy(bvb_sb[:], bps[:])
            for i in range(min(4, TT)):
                proj_v(i)
            proj_qk_group(0, 0, 0, act_copy=True)
            if NSCH > 1:
                proj_qk_group(0, 0, 1, act_copy=True)
            if FT > 1:
                # Q feature tile 1's first query half fills the DMA hole
                # between xq groups 0-1 landing and K's first group
                proj_qk_group(0, 1, 0, act_copy=True)
                if NSCH > 1:
                    proj_qk_group(0, 1, 1, act_copy=True)
            proj_qk_group(1, 0, 0, act_copy=True, c0=0, cw=SCH // 2)
            pending_sts.append(st_tile(0, *head_args(0, 0)))
            if TT > 1:
                pending_sts.append(st_tile(1, *head_args(0, 0)))
            proj_qk_group(1, 0, 0, act_copy=True, c0=SCH // 2, cw=SCH // 2)

            # Sweep order: all pairs' first query half, then all pairs'
            # second half. The out-projection of the first half then has the
            # whole sh=1 sweep to spread over instead of trailing the sweep.
            # Window work drips (step, item): each window carries ~2.5us so
            # it stays ACT(exp)-paced; window 0 carries the xk/xv DMA tail
            # (remaining V tiles + K j=0 chunks, which gate its own score
            # and context steps anyway).
            seq = [
                (2 * p + z, sh)
                for sh in range(NSH)
                for p in range(NH // 2)
                for z in (0, 1)
            ]
            half_tiles = T2 // 128 if NSH == 2 else TT
            wins = [[] for _ in seq]
            if TT == 16 and FT == 3 and NSCH == 4 and NSH == 2:
                # (enqueue_step, deadline_step or None, item); deadlines
                # force emission of projection chunks before the score tiles
                # that read them (K chunk n -> score tile 4n: deadline 4n-3;
                # Q chunks by the end of the window preceding their readers)
                wins[0] = [
                    (0, None, ("v", 4)),
                    (1, 1, ("qk", (1, 0, 1))),
                    (1, None, ("v", 5)),
                    (2, None, ("v", 6)),
                    (3, None, ("v", 7)),
                    (4, 5, ("qk", (1, 0, 2))),
                    (5, None, ("v", 8)),
                    (6, None, ("v", 9)),
                    (7, None, ("v", 10)),
                    (8, 9, ("qk", (1, 0, 3))),
                    (8, None, ("v", 11)),
                    (9, None, ("v", 12)),
                    (10, None, ("v", 13)),
                    (11, None, ("v", 14)),
                    (12, None, ("v", 15)),
                ]
                wins[1] = [(3, 13, ("qk", (1, 1, 0))), (7, 13, ("qk", (1, 1, 1))),
                           (11, 15, ("qk", (0, 0, 2)))]
                wins[2] = [(3, 4, ("qk", (1, 1, 2))), (7, 8, ("qk", (1, 1, 3))),
                           (11, 13, ("qk", (0, 2, 0)))]
                wins[3] = [(3, 13, ("qk", (1, 2, 0))), (7, 13, ("qk", (1, 2, 1))),
                           (11, 13, ("qk", (0, 2, 1)))]
                wins[4] = [(3, 4, ("qk", (1, 2, 2))), (7, 8, ("qk", (1, 2, 3))),
                           (11, 15, ("qk", (0, 0, 3)))]
                wins[5] = [(4, 15, ("qk", (0, 1, 2))), (9, 15, ("qk", (0, 1, 3)))]
                wins[6] = [(4, 15, ("qk", (0, 2, 2))), (10, None, ("out", 0))]
                wins[7] = [(4, 15, ("qk", (0, 2, 3))), (10, None, ("out", 1))]
                wins[8] = [(4, None, ("out", 2)), (10, None, ("out", 3))]
                wins[9] = [(4, None, ("out", 4)), (10, None, ("out", 5))]
                wins[10] = [(4, None, ("out", 6)), (10, None, ("out", 7))]
            else:
                # generic fallback: everything dripped in dependency order
                # with same-step deadlines (conservative)
                rest = (
                    [("qk", (0, 0, n)) for n in range(2, NSCH)]
                    + [("qk", (1, 0, n)) for n in range(1, NSCH)]
                    + [("v", i) for i in range(8, TT)]
                    + [
                        ("qk", (t, j, n))
                        for j in range(1, FT)
                        for t in (1, 0)
                        for n in range(NSCH)
                        if (t, j, n) not in ((0, 1, 0), (0, 1, 1))
                    ]
                    + [("out", i) for i in range(half_tiles)]
                )
                for z, it in enumerate(rest):
                    wins[min(z // 2, len(seq) - 2)].append((z % 14, z % 14, it))

            pos = 0
            for wi, (h, sh) in enumerate(seq):
                nxt = seq[pos + 1] if pos + 1 < len(seq) else None
                last_win = wi == len(seq) - 1
                head(
                    h,
                    sh,
                    items=sorted(wins[wi], key=lambda x: x[0]),
                    nxt=nxt,
                    norm_chunks=4 if last_win else 1,
                    budget=900 if last_win else None,
                )
                pos += 1
            drain_dq(0, flush=True)
            # tail: out-projection of the second query half on alternating
            # PSUM pools (score/proj rings are idle now) so the accumulation
            # chains never wait on a copy; the last head's normalize was
            # split so the first chunk's tiles start early
            tail_pools = ((stpool, "st"), (ppool, "proj"))
            for i in range(half_tiles, TT - 1):
                outproj(i, pools=tail_pools, tail=True)
            if half_tiles < TT:
                outproj(TT - 1, rows=slice(0, 64), pools=tail_pools, tail=True)
                outproj(TT - 1, rows=slice(64, 128), pools=tail_pools, tail=True)

    nc.compile()
    return nc


def shard_inputs(query, key, value, wq, bq, wk, bk, wv, bv, wo):
    """Build the 8 per-core input maps (host-side cast/transpose/slice)."""
    in_maps = []
    xT = {}
    for b in range(B):
        xT[b] = (
            np.ascontiguousarray(query[b].T).astype(NP_BF16),
            np.ascontiguousarray(key[b].T).astype(NP_BF16),
            np.ascontiguousarray(value[b].T).astype(NP_BF16),
        )
    gw = {}
    for g in range(2):
        hs = slice(g * G, (g + 1) * G)
        gw[g] = dict(
            wq=np.ascontiguousarray(wq[hs].transpose(1, 0, 2).reshape(E, G * DH)).astype(NP_BF16),
            wk=np.ascontiguousarray(wk[hs].transpose(1, 0, 2).reshape(E, G * DH)).astype(NP_BF16),
            wv=np.ascontiguousarray(wv[hs].transpose(1, 0, 2).reshape(E, G * DH)).astype(NP_BF16),
            wo=np.ascontiguousarray(wo[g * G * DH : (g + 1) * G * DH, :]).astype(NP_BF16),
            # q/k biases feature-tile-major [128, FT] fp32 for the DVE
            # per-partition bias-add; v bias [1, FEAT] bf16 for its matmul
            bq=np.ascontiguousarray(bq[hs].reshape(-1, 128).T).astype(np.float32),
            bk=np.ascontiguousarray(bk[hs].reshape(-1, 128).T).astype(np.float32),
            bv=np.ascontiguousarray(bv[hs].reshape(1, G * DH)).astype(NP_BF16),
        )
    for c in range(N_CORES):
        b, g = c // 2, c % 2
        m = dict(xqT=xT[b][0], xkT=xT[b][1], xvT=xT[b][2])
        m.update(gw[g])
        in_maps.append(m)
    return in_maps


_CACHED_NC = None


def kernel(query, key, value, wq, bq, wk, bk, wv, bv, wo, bo):
    global _CACHED_NC
    query, key, value = (np.asarray(a, np.float32) for a in (query, key, value))
    wq, bq, wk, bk, wv, bv, wo, bo = (
        np.asarray(a, np.float32) for a in (wq, bq, wk, bk, wv, bv, wo, bo)
    )
    in_maps = shard_inputs(query, key, value, wq, bq, wk, bk, wv, bv, wo)
    if _CACHED_NC is None:
        _CACHED_NC = build_nc()
    res = run_bass_kernel_spmd(_CACHED_NC, in_maps, list(range(N_CORES)))
    out = np.empty((B, S, E), np.float32)
    for b in range(B):
        out[b] = (
            res.results[2 * b]["out"].astype(np.float32)
            + res.results[2 * b + 1]["out"].astype(np.float32)
            + bo[None, :]
        )
    return out
